# revision 1
# baseline (speedup 1.0000x reference)
"""Causal self-attention (B=2, T=2048, E=1024, 16 heads) on 8 TRN2 NeuronCores.

Sharding (Megatron-style, zero device-side collectives):
  core c in 0..7 -> batch b = c//4, head group hg = c%4 (4 heads, 256 head-dims).
  Each core computes, for its batch and its 4 heads:
    qT/kT = (w_q|w_k)^T x^T   (transposed layout: [head_dim, T])
    v     = x w_v             (natural layout: [T, head_dim], + ones column)
    sT    = kT^T-block matmuls -> [tk, tq] score blocks (causal blocks only)
    expS  = exp(sT/8) * mask  (no max-subtraction: scores are O(1) by construction)
    yT    = v_plus^T @ expS   -> [65, tq]; row 64 accumulates the softmax row-sums
    y_norm= yT[0:64] * broadcast(1/rowsum)   (broadcast via K=1 matmul on PE)
    out_c = y_norm^T w_proj[rows of its heads]  -> partial [T, E]
  Host: out[b] = sum of the 4 partials + b_proj + b_v @ w_proj.
  b_k is dropped (softmax is invariant to per-row constants); b_q is applied
  on-device via the activation bias port; b_v is folded into the output bias.

All matmuls run in float32r (TF32-like, 1 cycle/row at N>=256; ~1.5e-4 rel err).
"""

import os

import numpy as np

N_HEAD = 16
E = 1024
B, T = 2, 2048
HD = E // N_HEAD          # 64
N_CORES = 8
HPC = 4                   # heads per core
DJ = HPC * HD             # 256 head-dim columns per core
ET = E // 128             # 8  e-tiles
TT = T // 128             # 16 t-tiles
TC = T // 512             # 4  t-chunks
SCALE = 1.0 / np.sqrt(HD)  # 0.125

_STATE = {}

# When True, x / w_qkv / w_v are shipped and consumed in bfloat16 (halves the
# startup HBM stream that gates the whole pipeline); scores, PV and the
# projection still run in float32r on fp32-accumulated q/k/v.
QKV_BF16 = True
QKV_FP16 = True  # fp16 beats bf16 for unit-scale data (10 vs 7 mantissa bits)


def _build_nc(reps=1):
    import concourse.tile as tile
    from concourse import mybir
    from concourse.bacc import Bacc

    f32 = mybir.dt.float32
    f32r = mybir.dt.float32r
    xdt = (mybir.dt.float16 if QKV_FP16 else mybir.dt.bfloat16) if QKV_BF16 else f32r
    AF = mybir.ActivationFunctionType

    nc = Bacc()
    xT_d = nc.dram_tensor("xT", [E, T], xdt, kind="ExternalInput")
    wqk_d = nc.dram_tensor("wqk", [E, 2 * DJ], xdt, kind="ExternalInput")
    wv_d = nc.dram_tensor("wv", [E, DJ], xdt, kind="ExternalInput")
    wp_d = nc.dram_tensor("wp", [DJ, E], f32r, kind="ExternalInput")
    bq_d = nc.dram_tensor("bq", [128, 2], f32, kind="ExternalInput")
    mask_d = nc.dram_tensor("mask", [4, 128, 512], f32, kind="ExternalInput")
    ones4_d = nc.dram_tensor("ones4", [128, HPC, 1], f32r, kind="ExternalInput")
    out_d = nc.dram_tensor("out", [T, E], f32, kind="ExternalOutput")

    with tile.TileContext(nc) as tc:
        with (
            tc.tile_pool(name="xw", bufs=1) as xw,          # persistent inputs
            tc.tile_pool(name="qkv", bufs=1) as qkv,        # persistent qT/kT/v/yT
            tc.tile_pool(name="es", bufs=8) as esp,         # exp(score) blocks
            tc.tile_pool(name="nrm", bufs=3) as nrm,        # norm scratch
            tc.tile_pool(name="ob", bufs=3) as obp,         # output staging
            tc.tile_pool(name="ps", bufs=2, space="PSUM") as ps,
            tc.tile_pool(name="psy", bufs=2, space="PSUM") as psy,
        ):
          nmask = 0  # running index to split mask-multiplies DVE/GPSIMD
          for _rep in range(reps):
            # ---- load persistent inputs ----
            xT_sb = []
            wqk_sb = []
            wv_sb = []
            for et in range(ET):
                t = xw.tile([128, T], xdt, tag=f"xT{et}", name=f"xT{et}")
                nc.sync.dma_start(t[:], xT_d[128 * et : 128 * (et + 1), :])
                xT_sb.append(t)
                t = xw.tile([128, 2 * DJ], xdt, tag=f"wqk{et}", name=f"wqk{et}")
                nc.sync.dma_start(t[:], wqk_d[128 * et : 128 * (et + 1), :])
                wqk_sb.append(t)
                t = xw.tile([128, DJ], xdt, tag=f"wv{et}", name=f"wv{et}")
                nc.sync.dma_start(t[:], wv_d[128 * et : 128 * (et + 1), :])
                wv_sb.append(t)
            # small tensors via the GPSIMD (SWDGE) path so they arrive in
            # parallel with the big SP-issued streams; chunk-0 attention
            # needs the masks almost immediately.
            bq_sb = xw.tile([128, 2], f32, tag="bq", name="bq")
            nc.gpsimd.dma_start(bq_sb[:], bq_d[:])
            mask_sb = []
            for m in range(4):
                t = xw.tile([128, 512], f32, tag=f"mask{m}", name=f"mask{m}")
                nc.gpsimd.dma_start(t[:], mask_d[m])
                mask_sb.append(t)
            ones4_sb = xw.tile([128, HPC, 1], f32r, tag="ones4", name="ones4")
            nc.gpsimd.dma_start(ones4_sb[:], ones4_d[:])
            wp_sb = []
            for kt in range(2):
                t = xw.tile([128, E], f32r, tag=f"wp{kt}", name=f"wp{kt}")
                nc.gpsimd.dma_start(t[:], wp_d[128 * kt : 128 * (kt + 1), :])
                wp_sb.append(t)
            if reps > 1 and _rep > 0:
                # measurement builds: serialize reps by folding a read-back
                # sampling EVERY output t-tile of the previous rep into the
                # v ones-column (timing-only perturbation of ~1e-7)
                chain = xw.tile([128, TT, 4], f32, tag="chain", name="chain")
                nc.sync.dma_start(
                    chain[:],
                    out_d.rearrange("(n p) e -> p n e", p=128)[:, :, 0:4],
                )
                red = xw.tile([128, 1], f32, tag="red", name="red")
                nc.vector.tensor_reduce(
                    out=red[:], in_=chain[:], axis=mybir.AxisListType.XY,
                    op=mybir.AluOpType.add,
                )
                o4b = xw.tile([128, HPC, 1], f32r, tag="ones4b", name="ones4b")
                rs = xw.tile([128, 1], f32, tag="rs", name="rs")
                nc.vector.tensor_scalar_mul(rs[:], red[:], 1e-7)
                with nc.allow_low_precision(reason="timing chain"):
                    nc.vector.tensor_scalar_add(o4b[:], ones4_sb[:], rs[:])
                ones4_sb = o4b

            # persistent intermediates
            qT_sb = [qkv.tile([128, T], f32r, tag=f"qT{i}", name=f"qT{i}") for i in range(2)]
            kT_sb = [qkv.tile([128, T], f32r, tag=f"kT{i}", name=f"kT{i}") for i in range(2)]
            v_sb = [qkv.tile([128, HPC, HD + 1], f32r, tag=f"v{i}", name=f"v{i}") for i in range(TT)]
            yT_sb = [qkv.tile([128, T], f32r, tag=f"yT{i}", name=f"yT{i}") for i in range(2)]

            # ---- phase 1+2: qT / kT / v, emitted chunk-by-chunk so attention
            # on chunk 0 can start after ~1/4 of the projection work ----
            if True:
              for ci in range(TC):
                  for jt in range(4):      # 0,1 -> q ; 2,3 -> k
                      acc = ps.tile([128, 512], f32, tag="mm", name="acc_qk")
                      for et in range(ET):
                          nc.tensor.matmul(
                              acc[:],
                              wqk_sb[et][:, 128 * jt : 128 * (jt + 1)],
                              xT_sb[et][:, 512 * ci : 512 * (ci + 1)],
                              start=(et == 0),
                              stop=(et == ET - 1),
                          )
                      if jt < 2:
                          # q: add bias while copying out of PSUM (DVE; keep the
                          # ACT engine free for the exp stream)
                          nc.vector.tensor_scalar_add(
                              qT_sb[jt][:, 512 * ci : 512 * (ci + 1)],
                              acc[:],
                              bq_sb[:, jt : jt + 1],
                          )
                      else:
                          nc.vector.tensor_copy(
                              kT_sb[jt - 2][:, 512 * ci : 512 * (ci + 1)], acc[:]
                          )
                  for tt in range(4 * ci, 4 * ci + 4):
                      acc = ps.tile([128, DJ], f32, tag="mm", name="acc_v")
                      for et in range(ET):
                          nc.tensor.matmul(
                              acc[:],
                              xT_sb[et][:, 128 * tt : 128 * (tt + 1)],
                              wv_sb[et][:],
                              start=(et == 0),
                              stop=(et == ET - 1),
                          )
                      nc.vector.tensor_copy(
                          v_sb[tt][:, :, 0:HD],
                          acc[:].rearrange("p (h d) -> p h d", h=HPC),
                      )
                      nc.vector.tensor_copy(v_sb[tt][:, :, HD : HD + 1], ones4_sb[:])

              # ---- phase 3+4: attention (chunk-outer, head-PAIR inner) ----
              # The two heads sharing a kT tile are computed together: their
              # score blocks land in one [128, 1024] psum (head A cols 0:512,
              # head B cols 512:1024) so ONE exp covers both.
              for ci in (0, 2, 3, 1):
                  nj = 4 * ci + 4
                  for hp in range(2):           # head pair: heads 2hp, 2hp+1
                      kth = kT_sb[hp]
                      qth = qT_sb[hp]
                      ya = psy.tile([HD + 1, 512], f32, tag="y", name="ya")
                      yb = psy.tile([HD + 1, 512], f32, tag="y", name="yb")
                      for j in range(nj):
                          s2 = ps.tile([128, 1024], f32, tag="s2", name="s2")
                          nrep = 2 if os.environ.get("KERNEL_MM_X2") else 1
                          for _mrep in range(nrep):
                              for half in range(2):
                                  nc.tensor.matmul(
                                      s2[:, 512 * half : 512 * half + 512],
                                      kth[HD * half : HD * half + HD,
                                          128 * j : 128 * (j + 1)],
                                      qth[HD * half : HD * half + HD,
                                          512 * ci : 512 * (ci + 1)],
                                  )
                          es = esp.tile([128, 1024], f32r, tag="es", name="es")
                          nc.scalar.activation(
                              out=es[:], in_=s2[:], func=AF.Exp, scale=float(SCALE)
                          )
                          if os.environ.get("KERNEL_EXP_X2"):
                              esx = esp.tile([128, 1024], f32r, tag="esx",
                                             name="esx", bufs=2)
                              nc.scalar.activation(
                                  out=esx[:], in_=s2[:], func=AF.Exp,
                                  scale=float(SCALE),
                              )
                          if j >= 4 * ci:
                              # mask both halves; mostly on the idle GPSIMD
                              m = mask_sb[j - 4 * ci]
                              for half in range(2):
                                  eng = nc.vector if nmask % 6 == 0 else nc.gpsimd
                                  eng.tensor_mul(
                                      es[:, 512 * half : 512 * half + 512],
                                      es[:, 512 * half : 512 * half + 512],
                                      m[:],
                                  )
                                  nmask += 1
                          nc.tensor.matmul(
                              ya[:], v_sb[j][:, 2 * hp, :], es[:, 0:512],
                              start=(j == 0), stop=(j == nj - 1),
                          )
                          nc.tensor.matmul(
                              yb[:], v_sb[j][:, 2 * hp + 1, :], es[:, 512:1024],
                              start=(j == 0), stop=(j == nj - 1),
                          )
                      # normalize: yT[0:64] * (1/rowsum); broadcast on GPSIMD
                      for half, yy in ((0, ya), (1, yb)):
                          rrow = nrm.tile([1, 512], f32, tag="rr", name="rrow")
                          nc.vector.reciprocal(rrow[:], yy[HD : HD + 1, :])
                          bs = nrm.tile([HD, 512], f32, tag="bs", name="bs")
                          nc.gpsimd.partition_broadcast(bs[:], rrow[:])
                          nc.vector.tensor_mul(
                              yT_sb[hp][HD * half : HD * half + HD,
                                        512 * ci : 512 * (ci + 1)],
                              yy[0:HD, :],
                              bs[:],
                          )
                  # projection for this chunk's 4 t-tiles
                  for tt in range(4 * ci, 4 * ci + 4):
                      ob = obp.tile([128, E], f32, tag="ob", name="ob")
                      for nk in range(2):
                          acc = ps.tile([128, 512], f32, tag="mm", name="acc_p")
                          for kt in range(2):
                              nc.tensor.matmul(
                                  acc[:],
                                  yT_sb[kt][:, 128 * tt : 128 * (tt + 1)],
                                  wp_sb[kt][:, 512 * nk : 512 * (nk + 1)],
                                  start=(kt == 0),
                                  stop=(kt == 1),
                              )
                          nc.vector.tensor_copy(ob[:, 512 * nk : 512 * (nk + 1)], acc[:])
                      nc.sync.dma_start(out_d[128 * tt : 128 * (tt + 1), :], ob[:])

    nc.finalize()
    return nc


def _host_constants():
    # diagonal causal masks: mask[m][r, c] = 1.0 if c >= r + 128*m else 0
    masks = np.zeros((4, 128, 512), dtype=np.float32)
    r = np.arange(128)[:, None]
    c = np.arange(512)[None, :]
    for m in range(4):
        masks[m] = (c >= r + 128 * m).astype(np.float32)
    ones4 = np.ones((128, HPC, 1), dtype=np.float32)
    return masks, ones4


def _make_in_maps(x, w_qkv, b_qkv):
    masks, ones4 = _host_constants()
    in_maps = []
    for c in range(N_CORES):
        b, hg = divmod(c, HPC)
        j0 = DJ * hg
        xT = np.ascontiguousarray(np.asarray(x[b], dtype=np.float32).T)
        wq = w_qkv[:, j0 : j0 + DJ]
        wk = w_qkv[:, E + j0 : E + j0 + DJ]
        wqk = np.ascontiguousarray(
            np.concatenate([wq, wk], axis=1), dtype=np.float32
        )
        wv = np.ascontiguousarray(w_qkv[:, 2 * E + j0 : 2 * E + j0 + DJ],
                                  dtype=np.float32)
        bq = np.ascontiguousarray(
            np.asarray(b_qkv[j0 : j0 + DJ], dtype=np.float32).reshape(2, 128).T
        )
        if QKV_BF16:
            if QKV_FP16:
                hdt = np.float16
            else:
                import ml_dtypes

                hdt = ml_dtypes.bfloat16
            xT = xT.astype(hdt)
            wqk = wqk.astype(hdt)
            wv = wv.astype(hdt)
        in_maps.append(
            {
                "xT": xT,
                "wqk": wqk,
                "wv": wv,
                "wp": None,  # filled below (needs w_proj)
                "bq": bq,
                "mask": masks,
                "ones4": ones4,
            }
        )
    return in_maps


def _get_exec():
    """Build the Bass module and a cached jitted SPMD callable (once)."""
    if "exec" in _STATE:
        return _STATE["exec"]

    import jax
    from concourse import bass2jax, mybir
    from jax.experimental.shard_map import shard_map
    from jax.sharding import Mesh, PartitionSpec

    nc = _build_nc()
    _STATE["nc"] = nc
    bass2jax.install_neuronx_cc_hook()

    partition_name = (
        nc.partition_id_tensor.name if nc.partition_id_tensor else None
    )
    in_names = []
    out_names = []
    out_avals = []
    zero_outs = []
    for alloc in nc.m.functions[0].allocations:
        if not isinstance(alloc, mybir.MemoryLocationSet):
            continue
        name = alloc.memorylocations[0].name
        if alloc.kind == "ExternalInput":
            if name != partition_name:
                in_names.append(name)
        elif alloc.kind == "ExternalOutput":
            shape = tuple(alloc.tensor_shape)
            dtype = mybir.dt.np(alloc.dtype)
            out_names.append(name)
            out_avals.append(jax.core.ShapedArray(shape, dtype))
            zero_outs.append(np.zeros(shape, dtype))
    n_params = len(in_names)
    all_names = in_names + out_names
    if partition_name is not None:
        all_names = all_names + [partition_name]

    def _make_body(k):
        def _body(*args):
            operands = list(args)
            if partition_name is not None:
                operands.append(bass2jax.partition_id_tensor())
            for _ in range(k):
                outs = bass2jax._bass_exec_p.bind(
                    *operands,
                    out_avals=tuple(out_avals),
                    in_names=tuple(all_names),
                    out_names=tuple(out_names),
                    lowering_input_output_aliases=(),
                    sim_require_finite=True,
                    sim_require_nnan=True,
                    nc=nc,
                )
            return tuple(outs)

        return _body

    devices = jax.devices()[:N_CORES]
    mesh = Mesh(np.asarray(devices), ("core",))
    n_all = n_params + len(out_names)

    def _make_sharded(k):
        return jax.jit(
            shard_map(
                _make_body(k),
                mesh=mesh,
                in_specs=(PartitionSpec("core"),) * n_all,
                out_specs=(PartitionSpec("core"),) * len(out_names),
                check_rep=False,
            ),
            keep_unused=True,
        )

    sharded = _make_sharded(1)

    state = {
        "make_sharded": _make_sharded,
        "jax": jax,
        "sharded": sharded,
        "in_names": in_names,
        "out_names": out_names,
        "out_avals": out_avals,
        "zeros_dev": [
            jax.device_put(
                np.zeros((N_CORES * z.shape[0], *z.shape[1:]), z.dtype)
            )
            for z in zero_outs
        ],
    }
    _STATE["exec"] = state
    return state


def _concat_inputs(in_maps):
    st = _get_exec()
    return [
        np.concatenate([np.asarray(in_maps[c][name]) for c in range(N_CORES)], axis=0)
        for name in st["in_names"]
    ]


def _run_device(concat_in):
    """concat_in: list of global (8*dim0, ...) arrays (np or jax). Returns
    list of per-core output dicts."""
    st = _get_exec()
    out_arrs = st["sharded"](*concat_in, *st["zeros_dev"])
    res = []
    for c in range(N_CORES):
        d = {}
        for i, name in enumerate(st["out_names"]):
            shp = st["out_avals"][i].shape
            d[name] = np.asarray(out_arrs[i]).reshape(N_CORES, *shp)[c]
        res.append(d)
    return res


def kernel(x, w_qkv, b_qkv, w_proj, b_proj):
    x = np.asarray(x, dtype=np.float32)
    w_qkv = np.asarray(w_qkv, dtype=np.float32)
    b_qkv = np.asarray(b_qkv, dtype=np.float32)
    w_proj = np.asarray(w_proj, dtype=np.float32)
    b_proj = np.asarray(b_proj, dtype=np.float32)

    in_maps = _make_in_maps(x, w_qkv, b_qkv)
    for c in range(N_CORES):
        _, hg = divmod(c, HPC)
        j0 = DJ * hg
        in_maps[c]["wp"] = np.ascontiguousarray(w_proj[j0 : j0 + DJ, :],
                                                dtype=np.float32)

    results = _run_device(_concat_inputs(in_maps))

    out = np.zeros((B, T, E), dtype=np.float32)
    for c in range(N_CORES):
        out[c // HPC] += results[c]["out"]
    # fold b_v through the projection; b_k cancels inside softmax
    bias = b_proj + b_qkv[2 * E :] @ w_proj
    out += bias[None, None, :]
    return out



# revision 32
# speedup vs baseline: 4.5637x; 4.5637x over previous
"""Causal self-attention (B=2, T=2048, E=1024, 16 heads) on 8 TRN2 NeuronCores.

Sharding (Megatron-style, zero device-side collectives):
  core c in 0..7 -> batch b = c//4, head group hg = c%4 (4 heads, 256 head-dims).
  Host: out[b] = sum of the 4 partials + b_proj + b_v @ w_proj.
  b_k is dropped (softmax is invariant to per-row constants); b_q is applied
  on-device; b_v is folded into the output bias.

Per-core kernel (all matmuls fp16 operands, fp32 PSUM accumulate):
  phase pipeline (software-pipelined by emission order; 4 t-chunks of 512):
    qkv(c):   qT/kT = w^T x^T  [d,t];  v = x w_v [t,d] (+ ones column)
    attn(c):  per head pair: block-causal scores sT[k,q] on PSUM, exp on ACT,
              PV computed Q-MAJOR: y'[128q, 65] += es_j^T @ v_j  (65 free
              cycles per block instead of 512), tensor_scalar normalize,
              PE-transpose back to yT[d, t].
    proj(c):  out[t,E] = yT^T w_proj, staged fp16, DMA out.
  qkv(c+1) and proj(c-1) matmuls are interleaved as fillers into attn(c)'s
  ACT-paced j-loop so the PE never idles waiting for exp.
"""

import numpy as np

N_HEAD = 16
E = 1024
B, T = 2, 2048
HD = E // N_HEAD          # 64
N_CORES = 8
HPC = 4                   # heads per core
DJ = HPC * HD             # 256 head-dim columns per core
ET = E // 128             # 8  e-tiles
TT = T // 128             # 16 t-tiles
TC = T // 512             # 4  t-chunks
SCALE = 1.0 / np.sqrt(HD)  # 0.125

_STATE = {}


def _build_nc(reps=1):
    import concourse.tile as tile
    from concourse import mybir
    from concourse.bacc import Bacc

    f32 = mybir.dt.float32
    f16 = mybir.dt.float16
    AF = mybir.ActivationFunctionType

    nc = Bacc()
    xT_d = nc.dram_tensor("xT", [E, T], f16, kind="ExternalInput")
    wqkv_d = nc.dram_tensor("wqkv", [E, 3 * DJ], f16, kind="ExternalInput")
    wp_d = nc.dram_tensor("wp", [DJ, E], f16, kind="ExternalInput")
    bq_d = nc.dram_tensor("bq", [128, 2], f32, kind="ExternalInput")
    mask_d = nc.dram_tensor("mask", [128, 2, 128], f16, kind="ExternalInput")
    ones4_d = nc.dram_tensor("ones4", [128, HPC, 1], f16, kind="ExternalInput")
    ident_d = nc.dram_tensor("ident", [128, 128], f16, kind="ExternalInput")
    out_d = nc.dram_tensor("out", [T, E], f16, kind="ExternalOutput")

    with tile.TileContext(nc) as tc:
        with (
            tc.tile_pool(name="xw", bufs=1) as xw,          # persistent inputs
            tc.tile_pool(name="qkv", bufs=1) as qkv,        # persistent qT/kT/v/yT
            tc.tile_pool(name="es", bufs=20) as esp,        # exp(score) blocks
            tc.tile_pool(name="nrm", bufs=2) as nrm,        # norm scratch
            tc.tile_pool(name="ob", bufs=3) as obp,         # output staging
            tc.tile_pool(name="ps2", bufs=2, space="PSUM") as ps2,   # scores
            tc.tile_pool(name="pyv", bufs=2, space="PSUM") as pyv,   # y' accum
            tc.tile_pool(name="pacc", bufs=2, space="PSUM") as pacc,  # qkv/proj/tr
        ):
            # ---- persistent input tiles ----
            xT_sb = [xw.tile([128, T], f16, tag=f"xT{et}", name=f"xT{et}")
                     for et in range(ET)]
            wqkv_sb = [xw.tile([128, 3 * DJ], f16, tag=f"wqkv{et}", name=f"wqkv{et}")
                       for et in range(ET)]
            # chunk-0 x and all w first: even e-tiles on SP, odd e-tiles on
            # the ACT DGE queue (idle until the first exp) so both streams
            # land in parallel.
            for et in range(ET):
                eng = nc.sync if et % 2 == 0 else nc.scalar
                eng.dma_start(wqkv_sb[et][:], wqkv_d[128 * et: 128 * (et + 1), :])
                eng.dma_start(xT_sb[et][:, 0:512],
                              xT_d[128 * et: 128 * (et + 1), 0:512])
            bq_sb = xw.tile([128, 2], f32, tag="bq", name="bq")
            nc.gpsimd.dma_start(bq_sb[:], bq_d[:])
            mask_sb = xw.tile([128, 2, 128], f16, tag="mask", name="mask")
            nc.gpsimd.dma_start(mask_sb[:], mask_d[:])
            ones4_sb = xw.tile([128, HPC, 1], f16, tag="ones4", name="ones4")
            nc.gpsimd.dma_start(ones4_sb[:], ones4_d[:])
            ident_sb = xw.tile([128, 128], f16, tag="ident", name="ident")
            nc.gpsimd.dma_start(ident_sb[:], ident_d[:])
            wp_sb = []
            for kt in range(2):
                t = xw.tile([128, E], f16, tag=f"wp{kt}", name=f"wp{kt}")
                nc.gpsimd.dma_start(t[:], wp_d[128 * kt: 128 * (kt + 1), :])
                wp_sb.append(t)
            # chunk-1 x on the SWDGE queue, chunks 2,3 on SP (needed later)
            for et in range(ET):
                nc.gpsimd.dma_start(xT_sb[et][:, 512:1024],
                                    xT_d[128 * et: 128 * (et + 1), 512:1024])
            for ci in (2, 3):
                for et in range(ET):
                    nc.sync.dma_start(
                        xT_sb[et][:, 512 * ci: 512 * (ci + 1)],
                        xT_d[128 * et: 128 * (et + 1), 512 * ci: 512 * (ci + 1)])

            # persistent intermediates (fp16)
            qT_sb = [qkv.tile([128, T], f16, tag=f"qT{i}", name=f"qT{i}") for i in range(2)]
            kT_sb = [qkv.tile([128, T], f16, tag=f"kT{i}", name=f"kT{i}") for i in range(2)]
            v_sb = [qkv.tile([128, HPC, HD + 1], f16, tag=f"v{i}", name=f"v{i}")
                    for i in range(TT)]
            yT_sb = [qkv.tile([128, T], f16, tag=f"yT{i}", name=f"yT{i}") for i in range(2)]

            # preload the Exp activation table while ACT is idle at startup
            warm = nrm.tile([128, 1], f32, tag="warm", name="warm")
            nc.vector.memset(warm[:], 0.0)
            nc.scalar.activation(out=warm[:], in_=warm[:], func=AF.Exp)

            # ---------- emission helpers ----------
            # Filler generators emit 1 "unit" (~0.2-0.5us of PE work) per
            # next() so they can be woven between attention j-iterations.
            def gen_qk(ci, jt):
                # jt: 0,1 -> q d-tiles; 2,3 -> k d-tiles. 4 units.
                acc = pacc.tile([128, 512], f32, tag="pacc", name="acc_qk")
                for et in range(ET):
                    nc.tensor.matmul(
                        acc[:],
                        wqkv_sb[et][:, 128 * jt: 128 * (jt + 1)],
                        xT_sb[et][:, 512 * ci: 512 * (ci + 1)],
                        start=(et == 0), stop=(et == ET - 1),
                    )
                    if et % 2 == 1 and et < ET - 1:
                        yield
                if jt < 2:
                    nc.vector.tensor_scalar_add(
                        qT_sb[jt][:, 512 * ci: 512 * (ci + 1)], acc[:],
                        bq_sb[:, jt: jt + 1])
                else:
                    nc.vector.tensor_copy(
                        kT_sb[jt - 2][:, 512 * ci: 512 * (ci + 1)], acc[:])
                yield

            def gen_v(tt):
                # 2 units
                acc = pacc.tile([128, DJ], f32, tag="pacc", name="acc_v")
                for et in range(ET):
                    nc.tensor.matmul(
                        acc[:],
                        xT_sb[et][:, 128 * tt: 128 * (tt + 1)],
                        wqkv_sb[et][:, 2 * DJ: 3 * DJ],
                        start=(et == 0), stop=(et == ET - 1),
                    )
                    if et == 3:
                        yield
                nc.vector.tensor_copy(
                    v_sb[tt][:, :, 0:HD],
                    acc[:].rearrange("p (h d) -> p h d", h=HPC))
                nc.vector.tensor_copy(v_sb[tt][:, :, HD: HD + 1], ones4_sb[:])
                yield

            def gen_proj_tile(tt, nk, copy_eng=None):
                # 1 unit
                acc = pacc.tile([128, 512], f32, tag="pacc", name="acc_p")
                for kt in range(2):
                    nc.tensor.matmul(
                        acc[:],
                        yT_sb[kt][:, 128 * tt: 128 * (tt + 1)],
                        wp_sb[kt][:, 512 * nk: 512 * (nk + 1)],
                        start=(kt == 0), stop=(kt == 1),
                    )
                ob = obp.tile([128, 512], f16, tag="ob", name="ob")
                if copy_eng is None:
                    nc.vector.tensor_copy(ob[:], acc[:])
                else:
                    copy_eng(ob[:], acc[:])
                nc.sync.dma_start(
                    out_d[128 * tt: 128 * (tt + 1), 512 * nk: 512 * (nk + 1)], ob[:])
                yield

            def gen_qkv_chunk(ci):
                yield from gen_qk(ci, 0)
                yield from gen_qk(ci, 2)
                for tt in range(4 * ci, 4 * ci + 4):
                    yield from gen_v(tt)
                yield from gen_qk(ci, 1)
                yield from gen_qk(ci, 3)

            def gen_proj_chunk(ci, alternate=False):
                i = 0
                for tt in range(4 * ci, 4 * ci + 4):
                    for nk in range(2):
                        eng = nc.scalar.copy if (alternate and i % 2) else None
                        yield from gen_proj_tile(tt, nk, copy_eng=eng)
                        i += 1

            class Fillers:
                """Filler work source. With `rate` set, pull() emits units at
                that average rate per call (credit accumulator), spreading a
                small supply over the whole consumer window."""

                def __init__(self, gens, rate=None):
                    self.gens = list(gens)
                    self.rate = rate
                    self.credit = 0.0

                def pull(self, n, weight=1.0):
                    if self.rate is not None:
                        self.credit += self.rate * weight
                        n = int(self.credit)
                    while n > 0 and self.gens:
                        try:
                            next(self.gens[0])
                            n -= 1
                            if self.rate is not None:
                                self.credit -= 1
                        except StopIteration:
                            self.gens.pop(0)

                def drain(self):
                    self.rate = None
                    self.pull(1 << 30)

            def emit_attn_pair(ci, hp, fillers, tail=False):
                """Attention for head pair hp on q-chunk ci; pulls filler work
                between j iterations to keep PE busy while ACT runs exp.
                PV accumulation for q-tile tq is one contiguous burst emitted
                when the diagonal block j=4ci+tq becomes available (one
                pending PSUM accumulation group per yps tile at a time)."""
                nj = 4 * ci + 4
                yps = [pyv.tile([128, 4, HD + 1], f32, tag="yps", name=f"yps{h}")
                       for h in range(2)]
                es_tiles = []
                rr = nrm.tile([128, 2, 4, 1], f32, tag="rr", name="rr")
                yn = nrm.tile([128, 2, 4, HD], f16, tag="yn", name="yn")
                ytr = None
                yT4 = yT_sb[hp].rearrange("p (c q) -> p c q", q=128)
                for j in range(nj):
                    d = j - 4 * ci  # >=0 on diagonal block-rows
                    q0 = max(d, 0) * 128  # first needed q column in chunk
                    s2 = ps2.tile([128, 2, 512], f32, tag="s2", name="s2")
                    for h in range(2):
                        nc.tensor.matmul(
                            s2[:, h, q0:512],
                            kT_sb[hp][HD * h: HD * h + HD, 128 * j: 128 * (j + 1)],
                            qT_sb[hp][HD * h: HD * h + HD,
                                      512 * ci + q0: 512 * (ci + 1)],
                        )
                    es = esp.tile([128, 2, 512], f16, tag="es", name="es")
                    es_tiles.append(es)
                    nc.scalar.activation(
                        out=es[:, :, q0:512], in_=s2[:, :, q0:512],
                        func=AF.Exp, scale=float(SCALE))
                    # filler matmuls run while ACT computes the exp
                    fillers.pull(2, weight=(1.0 if d < 0 else 0.3))
                    if d < 0:
                        continue
                    # triangular mask on the diagonal q-subtile (both heads);
                    # GPSIMD is idle and keeps this off DVE's critical path
                    nc.gpsimd.tensor_mul(
                        es[:, :, q0: q0 + 128], es[:, :, q0: q0 + 128],
                        mask_sb[:])
                    tq = d
                    for h in range(2):
                        for jj in range(j + 1):
                            nc.tensor.matmul(
                                yps[h][:, tq, :],
                                es_tiles[jj][:, h, 128 * tq: 128 * (tq + 1)],
                                v_sb[jj][:, 2 * hp + h, :],
                                start=(jj == 0), stop=(jj == j),
                            )
                    if not tail:
                        continue
                    # tail pair: normalize q-tile tq as soon as its rowsums
                    # are final (DVE/ACT split), transpose STAGGERED one
                    # diagonal-j behind so the PE never waits on the chain.
                    if ytr is None:
                        ytr = pacc.tile([128, 4, 128], f16, tag="pacc",
                                        name="ytr")
                    for h in range(2):
                        nc.vector.reciprocal(rr[:, h, tq],
                                             yps[h][:, tq, HD: HD + 1])
                        if h == 1:
                            nc.scalar.activation(
                                out=yn[:, h, tq], in_=yps[h][:, tq, 0:HD],
                                func=AF.Copy, scale=rr[:, h, tq])
                        else:
                            nc.vector.tensor_scalar_mul(
                                yn[:, h, tq], yps[h][:, tq, 0:HD], rr[:, h, tq])
                    if tq >= 1:
                        for h in range(2):
                            nc.tensor.transpose(
                                ytr[64 * h: 64 * h + 64, tq - 1, :],
                                yn[:, h, tq - 1], ident_sb[:])
                        nc.vector.tensor_copy(yT4[:, 4 * ci + tq - 1, :],
                                              ytr[:, tq - 1, :])
                    if tq >= 2:
                        for nk in range(2):
                            eng = nc.scalar.copy if nk else None
                            for _ in gen_proj_tile(4 * ci + tq - 2, nk,
                                                   copy_eng=eng):
                                pass
                if tail:
                    for h in range(2):
                        nc.tensor.transpose(
                            ytr[64 * h: 64 * h + 64, 3, :],
                            yn[:, h, 3], ident_sb[:])
                    nc.vector.tensor_copy(yT4[:, 4 * ci + 3, :], ytr[:, 3, :])
                    for tt in (4 * ci + 2, 4 * ci + 3):
                        for nk in range(2):
                            eng = nc.scalar.copy if nk else None
                            for _ in gen_proj_tile(tt, nk, copy_eng=eng):
                                pass
                else:
                    # normalize + transpose at pair end, fillers cover the
                    # DVE latency
                    for h in range(2):
                        nc.vector.reciprocal(rr[:, h], yps[h][:, :, HD: HD + 1])
                    ytr = pacc.tile([128, 4, 128], f16, tag="pacc", name="ytr")
                    for tq in range(4):
                        for h in range(2):
                            nc.vector.tensor_scalar_mul(
                                yn[:, h, tq], yps[h][:, tq, 0:HD], rr[:, h, tq])
                            nc.tensor.transpose(
                                ytr[64 * h: 64 * h + 64, tq, :],
                                yn[:, h, tq], ident_sb[:])
                        fillers.pull(1)
                    nc.vector.tensor_copy(yT4[:, 4 * ci: 4 * ci + 4, :], ytr[:])

            # ---------- pipeline ----------
            # chunk 0 qkv up front (DMA-paced)
            Fillers([gen_qkv_chunk(0)]).drain()
            # (supply units, pull slots) per chunk determine the ration rate.
            # Chunk-3 qkv is sliced by first-use: q(p0) before att(3) starts;
            # k(p0)/v before att(3) pair-0's diagonal js; q(p1) before pair 1;
            # k(p1) before pair-1's diagonal js.
            for ci in range(TC):
                nj = 4 * ci + 4
                slots = nj - 4 + 4 * 0.3 + 4  # weighted pull-slots per pair
                if ci < TC - 1:
                    if ci < TC - 2:
                        gens, units = [gen_qkv_chunk(ci + 1)], 24
                    else:
                        gens = [gen_qk(3, 0), gen_proj_chunk(0),
                                gen_proj_chunk(1)]
                        units = 20
                    fillers = Fillers(gens, rate=1.35 * units / (2 * slots))
                    emit_attn_pair(ci, 0, fillers)
                    emit_attn_pair(ci, 1, fillers)
                    fillers.drain()
                else:
                    f0 = Fillers([gen_qk(3, 2), gen_v(12), gen_v(13),
                                  gen_v(14), gen_v(15), gen_qk(3, 1)],
                                 rate=1.35 * 16 / slots)
                    emit_attn_pair(ci, 0, f0)
                    f0.drain()
                    f1 = Fillers([gen_qk(3, 3), gen_proj_chunk(2)],
                                 rate=1.35 * 12 / slots)
                    emit_attn_pair(ci, 1, f1, tail=True)
                    f1.drain()

    nc.finalize()
    return nc


def _host_constants():
    r = np.arange(128)[:, None]
    c = np.arange(128)[None, :]
    tri = (c >= r).astype(np.float16)           # allowed when q >= k
    mask = np.stack([tri, tri], axis=1)         # [128, 2, 128]
    ones4 = np.ones((128, HPC, 1), dtype=np.float16)
    ident = np.eye(128, dtype=np.float16)
    return mask, ones4, ident


def _make_in_maps(x, w_qkv, b_qkv):
    mask, ones4, ident = _host_constants()
    in_maps = []
    for c in range(N_CORES):
        b, hg = divmod(c, HPC)
        j0 = DJ * hg
        xT = np.ascontiguousarray(
            np.asarray(x[b], dtype=np.float32).T).astype(np.float16)
        wq = w_qkv[:, j0: j0 + DJ]
        wk = w_qkv[:, E + j0: E + j0 + DJ]
        wv = w_qkv[:, 2 * E + j0: 2 * E + j0 + DJ]
        wqkv = np.ascontiguousarray(
            np.concatenate([wq, wk, wv], axis=1), dtype=np.float32
        ).astype(np.float16)
        bq = np.ascontiguousarray(
            np.asarray(b_qkv[j0: j0 + DJ], dtype=np.float32).reshape(2, 128).T)
        in_maps.append(
            {
                "xT": xT,
                "wqkv": wqkv,
                "wp": None,  # filled in kernel() (needs w_proj)
                "bq": bq,
                "mask": mask,
                "ones4": ones4,
                "ident": ident,
            }
        )
    return in_maps


def _get_exec():
    """Build the Bass module and a cached jitted SPMD callable (once)."""
    if "exec" in _STATE:
        return _STATE["exec"]

    import jax
    from concourse import bass2jax, mybir
    from jax.experimental.shard_map import shard_map
    from jax.sharding import Mesh, PartitionSpec

    nc = _build_nc()
    _STATE["nc"] = nc
    bass2jax.install_neuronx_cc_hook()

    partition_name = (
        nc.partition_id_tensor.name if nc.partition_id_tensor else None
    )
    in_names = []
    out_names = []
    out_avals = []
    zero_outs = []
    for alloc in nc.m.functions[0].allocations:
        if not isinstance(alloc, mybir.MemoryLocationSet):
            continue
        name = alloc.memorylocations[0].name
        if alloc.kind == "ExternalInput":
            if name != partition_name:
                in_names.append(name)
        elif alloc.kind == "ExternalOutput":
            shape = tuple(alloc.tensor_shape)
            dtype = mybir.dt.np(alloc.dtype)
            out_names.append(name)
            out_avals.append(jax.core.ShapedArray(shape, dtype))
            zero_outs.append(np.zeros(shape, dtype))
    n_params = len(in_names)
    all_names = in_names + out_names
    if partition_name is not None:
        all_names = all_names + [partition_name]

    def _make_body(k):
        def _body(*args):
            operands = list(args)
            if partition_name is not None:
                operands.append(bass2jax.partition_id_tensor())
            for _ in range(k):
                outs = bass2jax._bass_exec_p.bind(
                    *operands,
                    out_avals=tuple(out_avals),
                    in_names=tuple(all_names),
                    out_names=tuple(out_names),
                    lowering_input_output_aliases=(),
                    sim_require_finite=True,
                    sim_require_nnan=True,
                    nc=nc,
                )
            return tuple(outs)

        return _body

    devices = jax.devices()[:N_CORES]
    mesh = Mesh(np.asarray(devices), ("core",))
    n_all = n_params + len(out_names)

    def _make_sharded(k):
        return jax.jit(
            shard_map(
                _make_body(k),
                mesh=mesh,
                in_specs=(PartitionSpec("core"),) * n_all,
                out_specs=(PartitionSpec("core"),) * len(out_names),
                check_rep=False,
            ),
            keep_unused=True,
        )

    sharded = _make_sharded(1)

    state = {
        "make_sharded": _make_sharded,
        "jax": jax,
        "sharded": sharded,
        "in_names": in_names,
        "out_names": out_names,
        "out_avals": out_avals,
        "zeros_dev": [
            jax.device_put(
                np.zeros((N_CORES * z.shape[0], *z.shape[1:]), z.dtype)
            )
            for z in zero_outs
        ],
    }
    _STATE["exec"] = state
    return state


def _concat_inputs(in_maps):
    st = _get_exec()
    return [
        np.concatenate([np.asarray(in_maps[c][name]) for c in range(N_CORES)], axis=0)
        for name in st["in_names"]
    ]


def _run_device(concat_in):
    """concat_in: list of global (8*dim0, ...) arrays (np or jax). Returns
    list of per-core output dicts."""
    st = _get_exec()
    out_arrs = st["sharded"](*concat_in, *st["zeros_dev"])
    res = []
    for c in range(N_CORES):
        d = {}
        for i, name in enumerate(st["out_names"]):
            shp = st["out_avals"][i].shape
            d[name] = np.asarray(out_arrs[i]).reshape(N_CORES, *shp)[c]
        res.append(d)
    return res


def kernel(x, w_qkv, b_qkv, w_proj, b_proj):
    x = np.asarray(x, dtype=np.float32)
    w_qkv = np.asarray(w_qkv, dtype=np.float32)
    b_qkv = np.asarray(b_qkv, dtype=np.float32)
    w_proj = np.asarray(w_proj, dtype=np.float32)
    b_proj = np.asarray(b_proj, dtype=np.float32)

    in_maps = _make_in_maps(x, w_qkv, b_qkv)
    for c in range(N_CORES):
        _, hg = divmod(c, HPC)
        j0 = DJ * hg
        in_maps[c]["wp"] = np.ascontiguousarray(
            w_proj[j0: j0 + DJ, :]).astype(np.float16)

    results = _run_device(_concat_inputs(in_maps))

    out = np.zeros((B, T, E), dtype=np.float32)
    for c in range(N_CORES):
        out[c // HPC] += results[c]["out"].astype(np.float32)
    # fold b_v through the projection; b_k cancels inside softmax
    bias = b_proj + b_qkv[2 * E:] @ w_proj
    out += bias[None, None, :]
    return out
